# revision 60
# baseline (speedup 1.0000x reference)
"""Trainium2 Bass kernel for nn_ConvUnit (cimu bit-sliced int8 conv2d).

Reference computation:
  xq = int8(trunc(clip(x, -128, 127)))                    # [32,128,56,56]
  for i in 0..7:
    bit_i = (xq >> i) & 1                                  # {0,1}
    c_i   = conv2d_valid(bit_i, W)                         # [32,128,54,54]
    q_i   = clip(round_half_even(c_i / 2), -128, 127) * 2
    y    += q_i * (2^i  if i < 7 else -128)
  y += bias

Strategy (8 NeuronCores, data-parallel over batch, 4 images/core).
Per-plane weight precision is chosen so the k_i-weighted rounding-flip
error stays well under the 2e-2 gate (measured ~1.39e-2 on the real
inputs, bit-stable across runs):
  * Planes 3-6: one shared fp16 stationary set W16 = fp16(W/2), a single
    matmul pass per tap (fp16's 10 mantissa bits vs bf16's 7 make the
    baseline's hi/lo pair unnecessary).  The per-plane scale k_i folds
    into the ACT stage: t = Copy(k_i*z + M_i) with M_i = 1.5*2^23*|k_i|;
    the RNE f32 add rounds z*k_i to a multiple of k_i ==
    k_i*round_half_even(c_i/2) (clip never fires; checked on host).
    DVE scalar_tensor_tensor fuses (t - M_i) + y.
  * Planes 0-2 (k=2,4,8): fp8e4m3 DoubleRowSwInterleave matmuls with the
    3 kw-taps parity-packed two-per-PE-cell: even/odd output columns are
    separate 243-col matmuls whose 16-bit rhs reads cover two adjacent
    input pixels -> 12 matmuls of 243 cols replace 9 of 486 (1.75x).
    Even/odd accumulate into separate psum halves (two groups, even
    fully first); the ACT pass re-interleaves via a strided source view.
    fp8 bits are repacked to 54-byte row pitch at col offsets 0/2 so
    every window is a contiguous 3-D AP.
  * Plane 7 (k=-256): fp16 hi A''=-2^17*fp16(W/2) (parity-split 243-col
    passes into the same psum halves) + fp8 parity lo residual, psum
    carries 512*z, ACT folds 2^-9.  delta ~2^-15.  The bias add rides
    the plane-7 init (tensor_scalar subtract-M-add-bias).
  * Conv as shifted matmuls over [9 rows x 54 cols] 2-D windows (no
    garbage columns), accumulating in PSUM; weights-outer over tile
    pairs in the parity paths keeps the 256-col LDWEIGHTS hidden.
  * Schedule: dummy matmuls warm the PE HAM clock gate during the DMA
    startup; img0's x loads in quarters so plane-7 matmuls (bit7 =
    (x<=-1), no trunc ladder needed) start ~12us in; img1's plane 7 runs
    right after img0's so the img0 trunc ladder hides under ~38us of
    matmul; the ladder for img i+1 is spread one DVE op per plane-step
    of img i to avoid blocking per-tile post-math in the DVE FIFO; bit
    planes are emitted with two-step lookahead.
Measured: 780974 ns (baseline) -> ~370700 ns, rel err 1.391e-2.
"""
import sys

sys.path.insert(0, "/opt/trn_rl_repo")

import numpy as np
import ml_dtypes

import concourse.bass as bass
import concourse.tile as tile
from concourse import bacc, mybir
from concourse import bass_utils

N_CORES = 8
B, C, H, W = 32, 128, 56, 56
HO, WO = 54, 54
BPC = B // N_CORES            # images per core
NPIX_IN = H * W               # 3136
HALF = 1568                   # img0 x/bit7 split point (28 rows)
ROWS_PER_TILE = 9
NTILES = HO // ROWS_PER_TILE  # 6
TILE_N = ROWS_PER_TILE * WO   # 486 <= 512 (one PSUM bank)
N_DUMMY = 16                  # HAM warmup matmuls during input DMA

MAGIC = 12582912.0            # 1.5 * 2^23: RNE(z + MAGIC) - MAGIC == rhe(z)
KSCALE = [float(2 << i) for i in range(7)] + [-256.0]

# weight block layout: [W16: taps 0-8][A'': 9]
NBLK = 18

# planes computed via fp8e4m3 DoubleRow matmuls with taps parity-packed
# two-per-PE-cell: 12 matmuls of 243 cols vs 9 of 486 (1.75x fewer cycles)
PARITY_PLANES = (0, 1, 2)
FP8_SCALE = 64.0              # w/2 * 64 centers weights in e4m3 range
NSET8 = 12                    # DoubleRow weight sets per parity plane pass

AluOp = mybir.AluOpType
ActFn = mybir.ActivationFunctionType
F32 = mybir.dt.float32
I32 = mybir.dt.int32
F16 = mybir.dt.float16
F8 = mybir.dt.float8e4
F8NP = ml_dtypes.float8_e4m3
DR = mybir.MatmulPerfMode.DoubleRowSwInterleave


def _prep_weights(weight: np.ndarray) -> np.ndarray:
    """-> [128ci, 18blk*128co] fp16 lhsT blocks: [W16 x9 taps][A'' x9]."""
    w2 = weight.astype(np.float32) * np.float32(0.5)   # [co, ci, kh, kw]
    w16 = w2.astype(np.float16)
    a = (w2 * np.float32(128.0)).astype(np.float16)
    app = (-1024.0 * a.astype(np.float32)).astype(np.float16)  # exact
    out = np.empty((C, 2, 9, C), dtype=np.float16)
    for s, src in enumerate((w16, app)):
        # [co, ci, kh, kw] -> [ci, tap, co]
        out[:, s] = src.transpose(1, 2, 3, 0).reshape(C, 9, C)
    return np.ascontiguousarray(out.reshape(C, NBLK * C))


def _parity_sets(wq: np.ndarray) -> np.ndarray:
    """DoubleRow parity weight sets -> [128ci, 12, 128co, 2slot] fp8.

    Output column c touches input bytes c+dw (dw=0..2).  With 16-bit
    aligned byte pairs (2q, 2q+1), per kernel row dh:
      even c=2q:   pair@2q   slots (w0, w1);  pair@2q+2 slots (w2, 0)
      odd  c=2q+1: pair@2q   slots (0,  w0);  pair@2q+2 slots (w1, w2)
    Set index = dh*4 + q with q in [Et0, Et1, Ot0, Ot1].
    DoubleRowSwInterleave layout: per partition row, co descending with
    (slot0, slot1) byte pairs interleaved: [A127 B127 A126 B126 ... B0].
    """
    out = np.zeros((C, NSET8, C, 2), dtype=F8NP)   # [ci, set, co_rev, slot]
    for dh in range(3):
        w0 = wq[:, :, dh, 0].T.astype(F8NP)        # [ci, co]
        w1 = wq[:, :, dh, 1].T.astype(F8NP)
        w2 = wq[:, :, dh, 2].T.astype(F8NP)
        for q, (s0, s1) in enumerate(
                [(w0, w1), (w2, None), (None, w0), (w1, w2)]):
            if s0 is not None:
                out[:, dh * 4 + q, ::-1, 0] = s0
            if s1 is not None:
                out[:, dh * 4 + q, ::-1, 1] = s1
    return out


def _prep_w8(weight: np.ndarray) -> np.ndarray:
    """[group0: planes 0-2 (w/2*64)][group1: plane-7 lo residual] fp8."""
    w2 = weight.astype(np.float32) * np.float32(0.5)
    a = (w2 * np.float32(128.0)).astype(np.float16)
    app = (-1024.0 * a.astype(np.float32)).astype(np.float32)
    r7 = -np.float32(2.0 ** 17) * w2 - app     # ~64*w2 scale, e4m3 range
    out = np.concatenate([_parity_sets(w2 * np.float32(FP8_SCALE)),
                          _parity_sets(r7)], axis=1)
    return np.ascontiguousarray(out.reshape(C, 2 * NSET8 * C * 2))


def _build(need_clip: bool):
    nc = bacc.Bacc("TRN2", target_bir_lowering=False, debug=False,
                   num_devices=N_CORES)
    xs = nc.dram_tensor("xs", [BPC, C, NPIX_IN], F32, kind="ExternalInput").ap()
    wt = nc.dram_tensor("wt", [C, NBLK * C], F16, kind="ExternalInput").ap()
    w8 = nc.dram_tensor("w8", [C, 2 * NSET8 * C * 2], F8,
                        kind="ExternalInput").ap()
    bs = nc.dram_tensor("bs", [C, 1], F32, kind="ExternalInput").ap()
    out = nc.dram_tensor("out", [BPC, C, HO, WO], F32, kind="ExternalOutput").ap()

    with tile.TileContext(nc) as tc:
        with (
            tc.tile_pool(name="spool", bufs=1) as spool,
            tc.tile_pool(name="wpool", bufs=1) as wpool,
            tc.tile_pool(name="cpool", bufs=1) as cpool,
            tc.tile_pool(name="xpool", bufs=3) as xpool,
            tc.tile_pool(name="tpool", bufs=1) as tpool,
            tc.tile_pool(name="xqpool", bufs=2) as xqpool,
            tc.tile_pool(name="b32pool", bufs=2) as b32pool,
            tc.tile_pool(name="bitpool", bufs=3) as bitpool,
            tc.tile_pool(name="bit8pool", bufs=4) as bit8pool,
            tc.tile_pool(name="ypool", bufs=2) as ypool,
            tc.tile_pool(name="upool", bufs=6) as upool,
            tc.tile_pool(name="psum", bufs=8, space="PSUM") as pspool,
        ):
            # ---- HAM warmup: dummy matmuls on zeroed scratch ----
            scratch = spool.tile([C, C + TILE_N], F16)
            nc.scalar.memzero(scratch[:])
            dps = pspool.tile([C, TILE_N], F32, tag="ps")
            for _ in range(N_DUMMY):
                nc.tensor.matmul(dps[:], scratch[:, :C],
                                 scratch[:, C:C + TILE_N],
                                 start=True, stop=True)

            wsb = wpool.tile([C, NBLK * C], F16)
            bsb = cpool.tile([C, 1], F32)
            xts = [xpool.tile([C, NPIX_IN], F32, tag="x", name=f"xt{i}")
                   for i in range(BPC)]
            # DMA order: img0 x first half, plane-7 weights (A'' + lo8), rest
            w8sb = wpool.tile([C, 2 * NSET8 * C * 2], F8)
            Q = 14 * W
            nc.sync.dma_start(xts[0][:, :Q], xs[0][:, :Q])
            nc.sync.dma_start(xts[0][:, Q:2 * Q], xs[0][:, Q:2 * Q])
            nc.sync.dma_start(wsb[:, 9 * C:], wt[:, 9 * C:])
            nc.sync.dma_start(w8sb[:, NSET8 * 2 * C:], w8[:, NSET8 * 2 * C:])
            nc.sync.dma_start(xts[0][:, 2 * Q:3 * Q], xs[0][:, 2 * Q:3 * Q])
            nc.sync.dma_start(xts[0][:, 3 * Q:], xs[0][:, 3 * Q:])
            nc.sync.dma_start(xts[1][:], xs[1])
            nc.sync.dma_start(wsb[:, :9 * C], wt[:, :9 * C])
            nc.sync.dma_start(w8sb[:, :NSET8 * 2 * C], w8[:, :NSET8 * 2 * C])
            nc.sync.dma_start(bsb[:], bs[:])
            nc.sync.dma_start(xts[2][:], xs[2])
            nc.sync.dma_start(xts[3][:], xs[3])

            bit = {}     # (img, plane) -> SBUF fp16 (or repacked fp8) tile
            bit8s = {}   # (img, 7) -> repacked fp8 bit7 for the lo pass
            xqs = {}     # img -> int32 xq tile
            yts = {}     # img -> y accumulator tile

            def emit_bit7(i, halves=False):
                # DVE is_le writes fp16 directly (no ACT convert); the fp8
                # repacks for the lo pass read the fp16 tensor
                bt = bitpool.tile([C, NPIX_IN], F16, tag="bit")
                b8 = bit8pool.tile([C, 2 * 54 * H], F8, tag="bit8",
                                   name="b8")
                btv = bt[:].rearrange("p (h w) -> p h w", w=W)
                rngs = [(0, 14), (14, 28), (28, 42), (42, 56)] if halves \
                    else [(0, 56)]
                for ra, rb in rngs:
                    a, b = ra * W, rb * W
                    nc.vector.tensor_scalar(bt[:, a:b], xts[i][:, a:b],
                                            -1.0, None, AluOp.is_le)
                    for t in (0, 1):
                        nc.scalar.copy(
                            b8[:, t * 54 * H + ra * 54:
                               t * 54 * H + rb * 54].rearrange(
                                "p (h w) -> p h w", w=54),
                            btv[:, ra:rb, 2 * t:2 * t + 54])
                bit[(i, 7)] = bt
                bit8s[(i, 7)] = b8

            def emit_bitlow(i, p):
                b32 = b32pool.tile([C, NPIX_IN], I32, tag="b32")
                nc.vector.tensor_scalar(b32[:], xqs[i][:], p, 1,
                                        AluOp.logical_shift_right,
                                        AluOp.bitwise_and)
                if p in PARITY_PLANES:
                    # repack at 54-byte row pitch, col offsets 0 and 2, so
                    # DoubleRow windows are contiguous 3-D APs [K, 2, 243]
                    bt = bit8pool.tile([C, 2 * 54 * H], F8, tag="bit8")
                    bsrc = b32[:].rearrange("p (h w) -> p h w", w=W)
                    for t in (0, 1):
                        nc.scalar.copy(
                            bt[:, t * 54 * H:(t + 1) * 54 * H].rearrange(
                                "p (h w) -> p h w", w=54),
                            bsrc[:, :, 2 * t:2 * t + 54])
                else:
                    bt = bitpool.tile([C, NPIX_IN], F16, tag="bit")
                    nc.scalar.copy(bt[:], b32[:])
                bit[(i, p)] = bt

            class Ladder:
                """xq = trunc(clip(x)) as int32, one op per emit_next()."""
                def __init__(self, img):
                    self.img = img
                    self.k = 0
                    self.at = None
                    self.st = None

                def emit_next(self):
                    xt = xts[self.img]
                    k = self.k
                    self.k += 1
                    if k == 0:
                        # c = min(max(x, -128), 127) in place; |c|, sign(c)
                        nc.vector.tensor_scalar(xt[:], xt[:], -128.0, 127.0,
                                                AluOp.max, AluOp.min)
                        self.at = tpool.tile([C, NPIX_IN], F32, tag="ta",
                                             name=f"at{self.img}")
                        nc.scalar.activation(self.at[:], xt[:], ActFn.Abs)
                        self.st = tpool.tile([C, NPIX_IN], F32, tag="ts",
                                             name=f"st{self.img}")
                        nc.scalar.activation(self.st[:], xt[:], ActFn.Sign)
                    elif k == 1:
                        # f = rhe(|c|)  (into xt)
                        nc.vector.tensor_scalar(xt[:], self.at[:], MAGIC,
                                                MAGIC, AluOp.add,
                                                AluOp.subtract)
                    elif k == 2:
                        # g = (f > |c|)  (into at)
                        nc.vector.tensor_tensor(self.at[:], xt[:], self.at[:],
                                                AluOp.is_gt)
                    elif k == 3:
                        # floor(|c|) = f - g
                        nc.vector.tensor_tensor(xt[:], xt[:], self.at[:],
                                                AluOp.subtract)
                    elif k == 4:
                        # trunc(c) = floor(|c|) * sign(c)
                        nc.vector.tensor_tensor(xt[:], xt[:], self.st[:],
                                                AluOp.mult)
                    elif k == 5:
                        xq = xqpool.tile([C, NPIX_IN], I32, tag="xq")
                        nc.vector.tensor_copy(xq[:], xt[:])
                        xqs[self.img] = xq

            # ---- prologue: img0 bit7 + ladder, img1 bit7 ----
            emit_bit7(0, halves=True)
            lad0 = Ladder(0)
            lad0.emit_next()        # clip + abs + sign
            lad0.emit_next()        # rhe
            emit_bit7(1)
            for _ in range(4):      # is_gt, sub, mult, xq
                lad0.emit_next()
            ladders = {i: Ladder(i) for i in range(1, BPC)}

            # ---- step sequence ----
            seq = ([(0, 7), (1, 7)]
                   + [(0, p) for p in range(7)] + [(2, 7)]
                   + [(1, p) for p in range(7)] + [(3, 7)]
                   + [(2, p) for p in range(7)]
                   + [(3, p) for p in range(7)])

            for n, (i, p) in enumerate(seq):
                # hosted ladder op for the next image (planes 0..5 host ops
                # 0..5).  The clip/abs/sign op (k==0) is emitted AFTER the
                # bit lookahead so the 2.8us ACT abs/sign don't delay bit
                # converts/repacks in the ACT FIFO; later ops (pure DVE,
                # including the xq the lookahead depends on) go first.
                host = ladders.get(i + 1) if p <= 5 else None
                if host is not None and host.k != 0:
                    host.emit_next()
                # two-step-lookahead bit emission
                for m in (n + 1, n + 2):
                    if m < len(seq) and seq[m] not in bit:
                        jq = seq[m]
                        if jq[1] == 7:
                            emit_bit7(jq[0])
                        else:
                            emit_bitlow(*jq)
                if host is not None and host.k == 0:
                    host.emit_next()

                if p == 7:
                    yts[i] = ypool.tile([C, HO * WO], F32, tag="y",
                                        name=f"yt{i}")
                yt = yts[i]
                bt = bit.pop((i, p))
                bv = bt[:].rearrange("p (h w) -> p h w", w=W)
                k = KSCALE[p]
                mag = MAGIC * abs(k)

                def post(j, ps, scale, deinter=False):
                    yv = yt[:, j * TILE_N:(j + 1) * TILE_N]
                    ut = upool.tile([C, TILE_N], F32, tag="u", name="ut")
                    if deinter:
                        # psum holds [even 243 | odd 243]; strided src view
                        # re-interleaves pixel parity during the ACT pass
                        src = ps[:].rearrange("p (two n) -> p n two", two=2)
                        dst = ut[:].rearrange("p (n two) -> p n two", two=2)
                    else:
                        src, dst = ps[:], ut[:]
                    nc.scalar.activation(dst, src, ActFn.Copy,
                                         bias=mag, scale=scale)
                    if need_clip:
                        lok, hik = (-128.0, 127.0) if k > 0 \
                            else (-127.0, 128.0)
                        nc.vector.tensor_scalar(
                            ut[:], ut[:],
                            mag + lok * abs(k), mag + hik * abs(k),
                            AluOp.max, AluOp.min)
                    # y = (t - M) + y   fused on DVE
                    nc.vector.scalar_tensor_tensor(
                        yv, ut[:], mag, yv, AluOp.subtract, AluOp.add)
                    if p == 6:
                        # last plane: per-tile writeout (bias was folded
                        # into the plane-7 init)
                        r0 = j * ROWS_PER_TILE
                        nc.sync.dma_start(
                            out[i][:, r0:r0 + ROWS_PER_TILE, :],
                            yt[:, j * TILE_N:(j + 1) * TILE_N].rearrange(
                                "p (h w) -> p h w", w=WO))

                if p == 7:
                    # first plane: fp16 hi (A'' = -2^17*fp16(w/2*128), split
                    # by output parity) + fp8 parity lo residual; psum holds
                    # 512*z in [even 243 | odd 243] halves; ACT folds 2^-9
                    b8 = bit8s.pop((i, 7))
                    bv2 = bt[:].rearrange("p (h q two) -> p h q two",
                                          h=H, two=2)
                    for half in range(NTILES // 2):
                        js = (2 * half, 2 * half + 1)
                        pss = [pspool.tile([C, TILE_N], F32, tag="ps",
                                           name=f"ps{j}") for j in js]
                        for par in range(2):
                            for tap in range(9):
                                dh, dw = tap // 3, tap % 3
                                qi, sl = divmod(par + dw, 2)
                                lw16 = wsb[:, (9 + tap) * C:(10 + tap) * C]
                                for ps, j in zip(pss, js):
                                    r0 = j * ROWS_PER_TILE
                                    rhs = bv2[:, r0 + dh:
                                              r0 + dh + ROWS_PER_TILE,
                                              qi:qi + 27, sl]
                                    nc.tensor.matmul(
                                        ps[:, par * 243:par * 243 + 243],
                                        lw16, rhs,
                                        start=(tap == 0), stop=False)
                            psets = (0, 1, 4, 5, 8, 9) if par == 0 \
                                else (2, 3, 6, 7, 10, 11)
                            for si, wset in enumerate(psets):
                                dh, q = wset // 4, wset % 4
                                t = q % 2
                                lw = w8sb[:, (NSET8 + wset) * 2 * C:
                                          (NSET8 + wset + 1) * 2 * C]
                                for ps, j in zip(pss, js):
                                    base = t * 54 * H \
                                        + (j * ROWS_PER_TILE + dh) * 54
                                    rhs = b8[:, base:base + TILE_N].rearrange(
                                        "p (n two) -> p two n", two=2)
                                    nc.tensor.matmul(
                                        ps[:, par * 243:par * 243 + 243],
                                        lw, rhs,
                                        start=False, stop=(si == 5),
                                        perf_mode=DR)
                        for ps, j in zip(pss, js):
                            yv = yt[:, j * TILE_N:(j + 1) * TILE_N]
                            ut = upool.tile([C, TILE_N], F32, tag="u",
                                            name="ut")
                            src = ps[:].rearrange("p (two n) -> p n two",
                                                  two=2)
                            dst = ut[:].rearrange("p (n two) -> p n two",
                                                  two=2)
                            nc.scalar.activation(dst, src, ActFn.Copy,
                                                 bias=mag, scale=1.0 / 512.0)
                            if need_clip:
                                nc.vector.tensor_scalar(yv, ut[:], mag, None,
                                                        AluOp.subtract)
                                nc.vector.tensor_scalar(yv, yv, -32512.0,
                                                        32768.0,
                                                        AluOp.max, AluOp.min)
                                nc.vector.tensor_scalar(yv, yv, bsb[:, 0:1],
                                                        None, AluOp.add)
                            else:
                                # fold the bias add into the first-plane
                                # write: (t - M) is small, + bias exact-safe
                                nc.vector.tensor_scalar(yv, ut[:], mag,
                                                        bsb[:, 0:1],
                                                        AluOp.subtract,
                                                        AluOp.add)
                    continue

                if p in PARITY_PLANES:
                    # fp8 DoubleRow, taps parity-packed 2/cell; weights-outer
                    # over tile pairs so the 256-col LDWEIGHTS stays hidden
                    for half in range(NTILES // 2):
                        js = (2 * half, 2 * half + 1)
                        pss = [pspool.tile([C, TILE_N], F32, tag="ps",
                                           name=f"ps{j}") for j in js]
                        # even sets fully first, then odd: two accumulation
                        # groups per psum tile (halves), no interleaved writes
                        for si, wset in enumerate((0, 1, 4, 5, 8, 9,
                                                   2, 3, 6, 7, 10, 11)):
                            dh, q = wset // 4, wset % 4
                            t, par = q % 2, q // 2
                            lw = w8sb[:, wset * 2 * C:(wset + 1) * 2 * C]
                            for ps, j in zip(pss, js):
                                base = t * 54 * H + (j * ROWS_PER_TILE + dh) * 54
                                rhs = bt[:, base:base + TILE_N].rearrange(
                                    "p (n two) -> p two n", two=2)
                                ov = ps[:, par * 243:par * 243 + 243]
                                nc.tensor.matmul(
                                    ov, lw, rhs,
                                    start=(si % 6 == 0), stop=(si % 6 == 5),
                                    perf_mode=DR)
                        for ps, j in zip(pss, js):
                            post(j, ps, k / FP8_SCALE, deinter=True)
                    continue

                # planes 2-6: single fp16 pass over shared W16
                for j in range(NTILES):
                    r0 = j * ROWS_PER_TILE
                    ps = pspool.tile([C, TILE_N], F32, tag="ps")
                    for tap in range(9):
                        dh, dw = tap // 3, tap % 3
                        nc.tensor.matmul(
                            ps[:],
                            wsb[:, tap * C:(tap + 1) * C],
                            bv[:, r0 + dh:r0 + dh + ROWS_PER_TILE,
                               dw:dw + WO],
                            start=(tap == 0),
                            stop=(tap == 8),
                        )
                    post(j, ps, KSCALE[p])

    nc.compile()
    return nc


_CACHE = {}


def _get_nc(need_clip: bool):
    if need_clip not in _CACHE:
        _CACHE[need_clip] = _build(need_clip)
    return _CACHE[need_clip]


def kernel(x: np.ndarray, weight: np.ndarray, bias: np.ndarray,
           _trace: bool = False):
    x = np.ascontiguousarray(x, dtype=np.float32)
    weight = np.ascontiguousarray(weight, dtype=np.float32)
    bias = np.ascontiguousarray(bias, dtype=np.float32)

    w_host = _prep_weights(weight)
    w8_host = _prep_w8(weight)
    # clip in the reference only fires if |conv/2| can reach 127.5
    need_clip = float(np.abs(weight).sum(axis=(1, 2, 3)).max()) * 0.5 >= 127.4
    nc = _get_nc(need_clip)

    bs_host = bias.reshape(C, 1)
    xr = x.reshape(B, C, NPIX_IN)
    in_maps = []
    for c in range(N_CORES):
        in_maps.append({
            "xs": np.ascontiguousarray(xr[c * BPC:(c + 1) * BPC]),
            "wt": w_host,
            "w8": w8_host,
            "bs": bs_host,
        })

    res = bass_utils.run_bass_kernel_spmd(
        nc, in_maps, core_ids=list(range(N_CORES)), trace=_trace)

    y = np.concatenate([res.results[c]["out"] for c in range(N_CORES)], axis=0)
    if _trace:
        kernel._last_results = res
    return y


if __name__ == "__main__":
    np.random.seed(0)
    x = (np.random.randn(B, C, H, W) * 60).astype(np.float32)
    w = (np.random.randn(C, C, 3, 3) * 0.05).astype(np.float32)
    b = np.random.randn(C).astype(np.float32)
    y = kernel(x, w, b)
    print("out", y.shape, y.dtype)


# revision 61
# speedup vs baseline: 1.0074x; 1.0074x over previous
"""Trainium2 Bass kernel for nn_ConvUnit (cimu bit-sliced int8 conv2d).

Reference computation:
  xq = int8(trunc(clip(x, -128, 127)))                    # [32,128,56,56]
  for i in 0..7:
    bit_i = (xq >> i) & 1                                  # {0,1}
    c_i   = conv2d_valid(bit_i, W)                         # [32,128,54,54]
    q_i   = clip(round_half_even(c_i / 2), -128, 127) * 2
    y    += q_i * (2^i  if i < 7 else -128)
  y += bias

Strategy (8 NeuronCores, data-parallel over batch, 4 images/core).
Per-plane weight precision is chosen so the k_i-weighted rounding-flip
error stays well under the 2e-2 gate (measured ~1.39e-2 on the real
inputs, bit-stable across runs):
  * Planes 3-6: one shared fp16 stationary set W16 = fp16(W/2), a single
    matmul pass per tap (fp16's 10 mantissa bits vs bf16's 7 make the
    baseline's hi/lo pair unnecessary).  The per-plane scale k_i folds
    into the ACT stage: t = Copy(k_i*z + M_i) with M_i = 1.5*2^23*|k_i|;
    the RNE f32 add rounds z*k_i to a multiple of k_i ==
    k_i*round_half_even(c_i/2) (clip never fires; checked on host).
    DVE scalar_tensor_tensor fuses (t - M_i) + y.
  * Planes 0-2 (k=2,4,8): fp8e4m3 DoubleRowSwInterleave matmuls with the
    3 kw-taps parity-packed two-per-PE-cell: even/odd output columns are
    separate 243-col matmuls whose 16-bit rhs reads cover two adjacent
    input pixels -> 12 matmuls of 243 cols replace 9 of 486 (1.75x).
    Even/odd accumulate into separate psum halves (two groups, even
    fully first); the ACT pass re-interleaves via a strided source view.
    fp8 bits are repacked to 54-byte row pitch at col offsets 0/2 so
    every window is a contiguous 3-D AP.
  * Plane 7 (k=-256): fp16 hi A''=-2^17*fp16(W/2) (parity-split 243-col
    passes into the same psum halves) + fp8 parity lo residual, psum
    carries 512*z, ACT folds 2^-9.  delta ~2^-15.  The bias add rides
    the plane-7 init (tensor_scalar subtract-M-add-bias).
  * Conv as shifted matmuls over [9 rows x 54 cols] 2-D windows (no
    garbage columns), accumulating in PSUM; weights-outer over tile
    pairs in the parity paths keeps the 256-col LDWEIGHTS hidden.
  * Schedule: dummy matmuls warm the PE HAM clock gate during the DMA
    startup; img0's x loads in quarters so plane-7 matmuls (bit7 =
    (x<=-1), no trunc ladder needed) start ~12us in; img1's plane 7 runs
    right after img0's so the img0 trunc ladder hides under ~38us of
    matmul; the ladder for img i+1 is spread one DVE op per plane-step
    of img i to avoid blocking per-tile post-math in the DVE FIFO; bit
    planes are emitted with two-step lookahead.
Measured: 780974 ns (baseline) -> ~370700 ns, rel err 1.391e-2.
"""
import sys

sys.path.insert(0, "/opt/trn_rl_repo")

import numpy as np
import ml_dtypes

import concourse.bass as bass
import concourse.tile as tile
from concourse import bacc, mybir
from concourse import bass_utils

N_CORES = 8
B, C, H, W = 32, 128, 56, 56
HO, WO = 54, 54
BPC = B // N_CORES            # images per core
NPIX_IN = H * W               # 3136
HALF = 1568                   # img0 x/bit7 split point (28 rows)
ROWS_PER_TILE = 9
NTILES = HO // ROWS_PER_TILE  # 6
TILE_N = ROWS_PER_TILE * WO   # 486 <= 512 (one PSUM bank)
N_DUMMY = 16                  # HAM warmup matmuls during input DMA

MAGIC = 12582912.0            # 1.5 * 2^23: RNE(z + MAGIC) - MAGIC == rhe(z)
KSCALE = [float(2 << i) for i in range(7)] + [-256.0]

# weight block layout: [W16: taps 0-8][A'': 9]
NBLK = 18

# planes computed via fp8e4m3 DoubleRow matmuls with taps parity-packed
# two-per-PE-cell: 12 matmuls of 243 cols vs 9 of 486 (1.75x fewer cycles)
PARITY_PLANES = (0, 1, 2)
FP8_SCALE = 64.0              # w/2 * 64 centers weights in e4m3 range
NSET8 = 12                    # DoubleRow weight sets per parity plane pass

AluOp = mybir.AluOpType
ActFn = mybir.ActivationFunctionType
F32 = mybir.dt.float32
I32 = mybir.dt.int32
F16 = mybir.dt.float16
F8 = mybir.dt.float8e4
F8NP = ml_dtypes.float8_e4m3
DR = mybir.MatmulPerfMode.DoubleRowSwInterleave


def _prep_weights(weight: np.ndarray) -> np.ndarray:
    """-> [128ci, 18blk*128co] fp16 lhsT blocks: [W16 x9 taps][A'' x9]."""
    w2 = weight.astype(np.float32) * np.float32(0.5)   # [co, ci, kh, kw]
    w16 = w2.astype(np.float16)
    a = (w2 * np.float32(128.0)).astype(np.float16)
    app = (-1024.0 * a.astype(np.float32)).astype(np.float16)  # exact
    out = np.empty((C, 2, 9, C), dtype=np.float16)
    for s, src in enumerate((w16, app)):
        # [co, ci, kh, kw] -> [ci, tap, co]
        out[:, s] = src.transpose(1, 2, 3, 0).reshape(C, 9, C)
    return np.ascontiguousarray(out.reshape(C, NBLK * C))


def _parity_sets(wq: np.ndarray) -> np.ndarray:
    """DoubleRow parity weight sets -> [128ci, 12, 128co, 2slot] fp8.

    Output column c touches input bytes c+dw (dw=0..2).  With 16-bit
    aligned byte pairs (2q, 2q+1), per kernel row dh:
      even c=2q:   pair@2q   slots (w0, w1);  pair@2q+2 slots (w2, 0)
      odd  c=2q+1: pair@2q   slots (0,  w0);  pair@2q+2 slots (w1, w2)
    Set index = dh*4 + q with q in [Et0, Et1, Ot0, Ot1].
    DoubleRowSwInterleave layout: per partition row, co descending with
    (slot0, slot1) byte pairs interleaved: [A127 B127 A126 B126 ... B0].
    """
    out = np.zeros((C, NSET8, C, 2), dtype=F8NP)   # [ci, set, co_rev, slot]
    for dh in range(3):
        w0 = wq[:, :, dh, 0].T.astype(F8NP)        # [ci, co]
        w1 = wq[:, :, dh, 1].T.astype(F8NP)
        w2 = wq[:, :, dh, 2].T.astype(F8NP)
        for q, (s0, s1) in enumerate(
                [(w0, w1), (w2, None), (None, w0), (w1, w2)]):
            if s0 is not None:
                out[:, dh * 4 + q, ::-1, 0] = s0
            if s1 is not None:
                out[:, dh * 4 + q, ::-1, 1] = s1
    return out


def _prep_w8(weight: np.ndarray) -> np.ndarray:
    """[group0: planes 0-2 (w/2*64)][group1: plane-7 lo residual] fp8."""
    w2 = weight.astype(np.float32) * np.float32(0.5)
    a = (w2 * np.float32(128.0)).astype(np.float16)
    app = (-1024.0 * a.astype(np.float32)).astype(np.float32)
    r7 = -np.float32(2.0 ** 17) * w2 - app     # ~64*w2 scale, e4m3 range
    out = np.concatenate([_parity_sets(w2 * np.float32(FP8_SCALE)),
                          _parity_sets(r7)], axis=1)
    return np.ascontiguousarray(out.reshape(C, 2 * NSET8 * C * 2))


def _build(need_clip: bool):
    nc = bacc.Bacc("TRN2", target_bir_lowering=False, debug=False,
                   num_devices=N_CORES)
    xs = nc.dram_tensor("xs", [BPC, C, NPIX_IN], F32, kind="ExternalInput").ap()
    wt = nc.dram_tensor("wt", [C, NBLK * C], F16, kind="ExternalInput").ap()
    w8 = nc.dram_tensor("w8", [C, 2 * NSET8 * C * 2], F8,
                        kind="ExternalInput").ap()
    bs = nc.dram_tensor("bs", [C, 1], F32, kind="ExternalInput").ap()
    out = nc.dram_tensor("out", [BPC, C, HO, WO], F32, kind="ExternalOutput").ap()

    with tile.TileContext(nc) as tc:
        with (
            tc.tile_pool(name="spool", bufs=1) as spool,
            tc.tile_pool(name="wpool", bufs=1) as wpool,
            tc.tile_pool(name="cpool", bufs=1) as cpool,
            tc.tile_pool(name="xpool", bufs=3) as xpool,
            tc.tile_pool(name="tpool", bufs=1) as tpool,
            tc.tile_pool(name="xqpool", bufs=2) as xqpool,
            tc.tile_pool(name="b32pool", bufs=2) as b32pool,
            tc.tile_pool(name="bitpool", bufs=3) as bitpool,
            tc.tile_pool(name="bit8pool", bufs=4) as bit8pool,
            tc.tile_pool(name="ypool", bufs=2) as ypool,
            tc.tile_pool(name="upool", bufs=6) as upool,
            tc.tile_pool(name="psum", bufs=8, space="PSUM") as pspool,
        ):
            # ---- HAM warmup: dummy matmuls on zeroed scratch ----
            scratch = spool.tile([C, C + TILE_N], F16)
            nc.scalar.memzero(scratch[:])
            dps = pspool.tile([C, TILE_N], F32, tag="ps")
            for _ in range(N_DUMMY):
                nc.tensor.matmul(dps[:], scratch[:, :C],
                                 scratch[:, C:C + TILE_N],
                                 start=True, stop=True)

            wsb = wpool.tile([C, NBLK * C], F16)
            bsb = cpool.tile([C, 1], F32)
            xts = [xpool.tile([C, NPIX_IN], F32, tag="x", name=f"xt{i}")
                   for i in range(BPC)]
            # DMA order: img0 x first half, plane-7 weights (A'' + lo8), rest
            w8sb = wpool.tile([C, 2 * NSET8 * C * 2], F8)
            Q = 14 * W
            nc.sync.dma_start(xts[0][:, :Q], xs[0][:, :Q])
            nc.sync.dma_start(xts[0][:, Q:2 * Q], xs[0][:, Q:2 * Q])
            nc.sync.dma_start(wsb[:, 9 * C:], wt[:, 9 * C:])
            nc.sync.dma_start(w8sb[:, NSET8 * 2 * C:], w8[:, NSET8 * 2 * C:])
            nc.sync.dma_start(xts[0][:, 2 * Q:3 * Q], xs[0][:, 2 * Q:3 * Q])
            nc.sync.dma_start(xts[0][:, 3 * Q:], xs[0][:, 3 * Q:])
            nc.sync.dma_start(xts[1][:], xs[1])
            nc.sync.dma_start(wsb[:, :9 * C], wt[:, :9 * C])
            nc.sync.dma_start(w8sb[:, :NSET8 * 2 * C], w8[:, :NSET8 * 2 * C])
            nc.sync.dma_start(bsb[:], bs[:])
            nc.sync.dma_start(xts[2][:], xs[2])
            nc.sync.dma_start(xts[3][:], xs[3])

            bit = {}     # (img, plane) -> SBUF fp16 (or repacked fp8) tile
            bit8s = {}   # (img, 7) -> repacked fp8 bit7 for the lo pass
            xqs = {}     # img -> int32 xq tile
            yts = {}     # img -> y accumulator tile

            def emit_bit7(i, halves=False):
                b7f = b32pool.tile([C, NPIX_IN], F32, tag="b32")
                bt = bitpool.tile([C, NPIX_IN], F16, tag="bit")
                b8 = bit8pool.tile([C, 2 * 54 * H], F8, tag="bit8",
                                   name="b8")
                b7v = b7f[:].rearrange("p (h w) -> p h w", w=W)
                rngs = [(0, 14), (14, 28), (28, 42), (42, 56)] if halves \
                    else [(0, 56)]
                for ra, rb in rngs:
                    a, b = ra * W, rb * W
                    nc.vector.tensor_scalar(b7f[:, a:b], xts[i][:, a:b],
                                            -1.0, None, AluOp.is_le)
                    nc.scalar.copy(bt[:, a:b], b7f[:, a:b])
                    for t in (0, 1):
                        nc.scalar.copy(
                            b8[:, t * 54 * H + ra * 54:
                               t * 54 * H + rb * 54].rearrange(
                                "p (h w) -> p h w", w=54),
                            b7v[:, ra:rb, 2 * t:2 * t + 54])
                bit[(i, 7)] = bt
                bit8s[(i, 7)] = b8

            def emit_bitlow(i, p):
                b32 = b32pool.tile([C, NPIX_IN], I32, tag="b32")
                nc.vector.tensor_scalar(b32[:], xqs[i][:], p, 1,
                                        AluOp.logical_shift_right,
                                        AluOp.bitwise_and)
                if p in PARITY_PLANES:
                    # repack at 54-byte row pitch, col offsets 0 and 2, so
                    # DoubleRow windows are contiguous 3-D APs [K, 2, 243]
                    bt = bit8pool.tile([C, 2 * 54 * H], F8, tag="bit8")
                    bsrc = b32[:].rearrange("p (h w) -> p h w", w=W)
                    for t in (0, 1):
                        nc.scalar.copy(
                            bt[:, t * 54 * H:(t + 1) * 54 * H].rearrange(
                                "p (h w) -> p h w", w=54),
                            bsrc[:, :, 2 * t:2 * t + 54])
                else:
                    bt = bitpool.tile([C, NPIX_IN], F16, tag="bit")
                    nc.scalar.copy(bt[:], b32[:])
                bit[(i, p)] = bt

            class Ladder:
                """xq = trunc(clip(x)) as int32, one op per emit_next()."""
                def __init__(self, img):
                    self.img = img
                    self.k = 0
                    self.at = None
                    self.st = None

                def emit_next(self):
                    xt = xts[self.img]
                    k = self.k
                    self.k += 1
                    if k == 0:
                        # c = min(max(x, -128), 127) in place; |c|, sign(c)
                        nc.vector.tensor_scalar(xt[:], xt[:], -128.0, 127.0,
                                                AluOp.max, AluOp.min)
                        self.at = tpool.tile([C, NPIX_IN], F32, tag="ta",
                                             name=f"at{self.img}")
                        nc.scalar.activation(self.at[:], xt[:], ActFn.Abs)
                        self.st = tpool.tile([C, NPIX_IN], F32, tag="ts",
                                             name=f"st{self.img}")
                        nc.scalar.activation(self.st[:], xt[:], ActFn.Sign)
                    elif k == 1:
                        # f = rhe(|c|)  (into xt)
                        nc.vector.tensor_scalar(xt[:], self.at[:], MAGIC,
                                                MAGIC, AluOp.add,
                                                AluOp.subtract)
                    elif k == 2:
                        # g = (f > |c|)  (into at)
                        nc.vector.tensor_tensor(self.at[:], xt[:], self.at[:],
                                                AluOp.is_gt)
                    elif k == 3:
                        # floor(|c|) = f - g
                        nc.vector.tensor_tensor(xt[:], xt[:], self.at[:],
                                                AluOp.subtract)
                    elif k == 4:
                        # trunc(c) = floor(|c|) * sign(c)
                        nc.vector.tensor_tensor(xt[:], xt[:], self.st[:],
                                                AluOp.mult)
                    elif k == 5:
                        xq = xqpool.tile([C, NPIX_IN], I32, tag="xq")
                        nc.vector.tensor_copy(xq[:], xt[:])
                        xqs[self.img] = xq

            # ---- prologue: img0 bit7 + ladder, img1 bit7 ----
            emit_bit7(0, halves=True)
            lad0 = Ladder(0)
            lad0.emit_next()        # clip + abs + sign
            lad0.emit_next()        # rhe
            emit_bit7(1)
            for _ in range(4):      # is_gt, sub, mult, xq
                lad0.emit_next()
            ladders = {i: Ladder(i) for i in range(1, BPC)}

            # ---- step sequence ----
            seq = ([(0, 7), (1, 7)]
                   + [(0, p) for p in range(7)] + [(2, 7)]
                   + [(1, p) for p in range(7)] + [(3, 7)]
                   + [(2, p) for p in range(7)]
                   + [(3, p) for p in range(7)])

            for n, (i, p) in enumerate(seq):
                # hosted ladder op for the next image (planes 0..5 host ops
                # 0..5).  The clip/abs/sign op (k==0) is emitted AFTER the
                # bit lookahead so the 2.8us ACT abs/sign don't delay bit
                # converts/repacks in the ACT FIFO; later ops (pure DVE,
                # including the xq the lookahead depends on) go first.
                host = ladders.get(i + 1) if p <= 5 else None
                if host is not None and host.k != 0:
                    host.emit_next()
                # two-step-lookahead bit emission
                for m in (n + 1, n + 2):
                    if m < len(seq) and seq[m] not in bit:
                        jq = seq[m]
                        if jq[1] == 7:
                            emit_bit7(jq[0])
                        else:
                            emit_bitlow(*jq)
                if host is not None and host.k == 0:
                    host.emit_next()

                if p == 7:
                    yts[i] = ypool.tile([C, HO * WO], F32, tag="y",
                                        name=f"yt{i}")
                yt = yts[i]
                bt = bit.pop((i, p))
                bv = bt[:].rearrange("p (h w) -> p h w", w=W)
                k = KSCALE[p]
                mag = MAGIC * abs(k)

                def post(j, ps, scale, deinter=False):
                    yv = yt[:, j * TILE_N:(j + 1) * TILE_N]
                    ut = upool.tile([C, TILE_N], F32, tag="u", name="ut")
                    if deinter:
                        # psum holds [even 243 | odd 243]; strided src view
                        # re-interleaves pixel parity during the ACT pass
                        src = ps[:].rearrange("p (two n) -> p n two", two=2)
                        dst = ut[:].rearrange("p (n two) -> p n two", two=2)
                    else:
                        src, dst = ps[:], ut[:]
                    nc.scalar.activation(dst, src, ActFn.Copy,
                                         bias=mag, scale=scale)
                    if need_clip:
                        lok, hik = (-128.0, 127.0) if k > 0 \
                            else (-127.0, 128.0)
                        nc.vector.tensor_scalar(
                            ut[:], ut[:],
                            mag + lok * abs(k), mag + hik * abs(k),
                            AluOp.max, AluOp.min)
                    # y = (t - M) + y   fused on DVE
                    nc.vector.scalar_tensor_tensor(
                        yv, ut[:], mag, yv, AluOp.subtract, AluOp.add)
                    if p == 6:
                        # last plane: per-tile writeout (bias was folded
                        # into the plane-7 init)
                        r0 = j * ROWS_PER_TILE
                        nc.sync.dma_start(
                            out[i][:, r0:r0 + ROWS_PER_TILE, :],
                            yt[:, j * TILE_N:(j + 1) * TILE_N].rearrange(
                                "p (h w) -> p h w", w=WO))

                if p == 7:
                    # first plane: fp16 hi (A'' = -2^17*fp16(w/2*128), split
                    # by output parity) + fp8 parity lo residual; psum holds
                    # 512*z in [even 243 | odd 243] halves; ACT folds 2^-9
                    b8 = bit8s.pop((i, 7))
                    bv2 = bt[:].rearrange("p (h q two) -> p h q two",
                                          h=H, two=2)
                    for half in range(NTILES // 2):
                        js = (2 * half, 2 * half + 1)
                        pss = [pspool.tile([C, TILE_N], F32, tag="ps",
                                           name=f"ps{j}") for j in js]
                        for par in range(2):
                            for tap in range(9):
                                dh, dw = tap // 3, tap % 3
                                qi, sl = divmod(par + dw, 2)
                                lw16 = wsb[:, (9 + tap) * C:(10 + tap) * C]
                                for ps, j in zip(pss, js):
                                    r0 = j * ROWS_PER_TILE
                                    rhs = bv2[:, r0 + dh:
                                              r0 + dh + ROWS_PER_TILE,
                                              qi:qi + 27, sl]
                                    nc.tensor.matmul(
                                        ps[:, par * 243:par * 243 + 243],
                                        lw16, rhs,
                                        start=(tap == 0), stop=False)
                            psets = (0, 1, 4, 5, 8, 9) if par == 0 \
                                else (2, 3, 6, 7, 10, 11)
                            for si, wset in enumerate(psets):
                                dh, q = wset // 4, wset % 4
                                t = q % 2
                                lw = w8sb[:, (NSET8 + wset) * 2 * C:
                                          (NSET8 + wset + 1) * 2 * C]
                                for ps, j in zip(pss, js):
                                    base = t * 54 * H \
                                        + (j * ROWS_PER_TILE + dh) * 54
                                    rhs = b8[:, base:base + TILE_N].rearrange(
                                        "p (n two) -> p two n", two=2)
                                    nc.tensor.matmul(
                                        ps[:, par * 243:par * 243 + 243],
                                        lw, rhs,
                                        start=False, stop=(si == 5),
                                        perf_mode=DR)
                        for ps, j in zip(pss, js):
                            yv = yt[:, j * TILE_N:(j + 1) * TILE_N]
                            ut = upool.tile([C, TILE_N], F32, tag="u",
                                            name="ut")
                            src = ps[:].rearrange("p (two n) -> p n two",
                                                  two=2)
                            dst = ut[:].rearrange("p (n two) -> p n two",
                                                  two=2)
                            nc.scalar.activation(dst, src, ActFn.Copy,
                                                 bias=mag, scale=1.0 / 512.0)
                            if need_clip:
                                nc.vector.tensor_scalar(yv, ut[:], mag, None,
                                                        AluOp.subtract)
                                nc.vector.tensor_scalar(yv, yv, -32512.0,
                                                        32768.0,
                                                        AluOp.max, AluOp.min)
                                nc.vector.tensor_scalar(yv, yv, bsb[:, 0:1],
                                                        None, AluOp.add)
                            else:
                                # fold the bias add into the first-plane
                                # write: (t - M) is small, + bias exact-safe
                                nc.vector.tensor_scalar(yv, ut[:], mag,
                                                        bsb[:, 0:1],
                                                        AluOp.subtract,
                                                        AluOp.add)
                    continue

                if p in PARITY_PLANES:
                    # fp8 DoubleRow, taps parity-packed 2/cell; weights-outer
                    # over tile pairs so the 256-col LDWEIGHTS stays hidden
                    for half in range(NTILES // 2):
                        js = (2 * half, 2 * half + 1)
                        pss = [pspool.tile([C, TILE_N], F32, tag="ps",
                                           name=f"ps{j}") for j in js]
                        # even sets fully first, then odd: two accumulation
                        # groups per psum tile (halves), no interleaved writes
                        for si, wset in enumerate((0, 1, 4, 5, 8, 9,
                                                   2, 3, 6, 7, 10, 11)):
                            dh, q = wset // 4, wset % 4
                            t, par = q % 2, q // 2
                            lw = w8sb[:, wset * 2 * C:(wset + 1) * 2 * C]
                            for ps, j in zip(pss, js):
                                base = t * 54 * H + (j * ROWS_PER_TILE + dh) * 54
                                rhs = bt[:, base:base + TILE_N].rearrange(
                                    "p (n two) -> p two n", two=2)
                                ov = ps[:, par * 243:par * 243 + 243]
                                nc.tensor.matmul(
                                    ov, lw, rhs,
                                    start=(si % 6 == 0), stop=(si % 6 == 5),
                                    perf_mode=DR)
                        for ps, j in zip(pss, js):
                            post(j, ps, k / FP8_SCALE, deinter=True)
                    continue

                # planes 2-6: single fp16 pass over shared W16
                for j in range(NTILES):
                    r0 = j * ROWS_PER_TILE
                    ps = pspool.tile([C, TILE_N], F32, tag="ps")
                    for tap in range(9):
                        dh, dw = tap // 3, tap % 3
                        nc.tensor.matmul(
                            ps[:],
                            wsb[:, tap * C:(tap + 1) * C],
                            bv[:, r0 + dh:r0 + dh + ROWS_PER_TILE,
                               dw:dw + WO],
                            start=(tap == 0),
                            stop=(tap == 8),
                        )
                    post(j, ps, KSCALE[p])

    nc.compile()
    return nc


_CACHE = {}


def _get_nc(need_clip: bool):
    if need_clip not in _CACHE:
        _CACHE[need_clip] = _build(need_clip)
    return _CACHE[need_clip]


def kernel(x: np.ndarray, weight: np.ndarray, bias: np.ndarray,
           _trace: bool = False):
    x = np.ascontiguousarray(x, dtype=np.float32)
    weight = np.ascontiguousarray(weight, dtype=np.float32)
    bias = np.ascontiguousarray(bias, dtype=np.float32)

    w_host = _prep_weights(weight)
    w8_host = _prep_w8(weight)
    # clip in the reference only fires if |conv/2| can reach 127.5
    need_clip = float(np.abs(weight).sum(axis=(1, 2, 3)).max()) * 0.5 >= 127.4
    nc = _get_nc(need_clip)

    bs_host = bias.reshape(C, 1)
    xr = x.reshape(B, C, NPIX_IN)
    in_maps = []
    for c in range(N_CORES):
        in_maps.append({
            "xs": np.ascontiguousarray(xr[c * BPC:(c + 1) * BPC]),
            "wt": w_host,
            "w8": w8_host,
            "bs": bs_host,
        })

    res = bass_utils.run_bass_kernel_spmd(
        nc, in_maps, core_ids=list(range(N_CORES)), trace=_trace)

    y = np.concatenate([res.results[c]["out"] for c in range(N_CORES)], axis=0)
    if _trace:
        kernel._last_results = res
    return y


if __name__ == "__main__":
    np.random.seed(0)
    x = (np.random.randn(B, C, H, W) * 60).astype(np.float32)
    w = (np.random.randn(C, C, 3, 3) * 0.05).astype(np.float32)
    b = np.random.randn(C).astype(np.float32)
    y = kernel(x, w, b)
    print("out", y.shape, y.dtype)
